# revision 54
# baseline (speedup 1.0000x reference)
"""Masked attention-weight kernel (dense_transformer) for 8 TRN2 NeuronCores.

Computes, for inputs query/key [32,1024,512] f32, masks [32,1024] i32:
    q = relu(query @ Wq + bq); k = relu(key @ Wk + bk)
    w = softmax((q @ k^T)/sqrt(512) + key_mask_additive) * query_mask
Output: [32, 1024, 1024] f32.

Strategy: data-parallel over batch (4 batches/core, no collectives) PLUS
host-side mask compaction.  Masked key columns have weight exactly 0 in the
reference (exp(-1e9) underflows) and masked query rows are zeroed, so the
host gathers only the valid ~512 query rows / key columns per batch, pads
them to a fixed NQP/NKP (multiple of 64, 576 for this data), and the device
runs dense attention on the compacted [NQP, NKP] problem -- ~2.4x fewer
matmul cycles than the full [1024,1024].  The host scatters the compact
bf16 output back into a zero-filled full-size f32 array.

Padded key columns are all-zero inputs, so (with zero bias -- true for this
problem) their projected features are 0, their logits are 0, and each
contributes exp(0)=1 to the softmax row-sum; the device subtracts the
host-provided pad count from the row-sum before taking the reciprocal.
If the key bias were nonzero the host instead ships an additive -1e4
column mask applied to the projected k (use_mask variant).

Per-core pipeline, per batch (all matmuls bf16 with f32 PSUM):
  1. kTm[e,j] = relu(Wk.T @ keyT + bk): PE matmuls in (512,48) psum-bank
     chunks -> relu+bias epilogue (wide chunks alternate DVE/ACT, ~740ns
     each since psum reads are 1 elem/cycle/lane on both; narrow on DVE).
  2. qT[e,i] likewise.
  3. Per 128-row block: S = qT.T @ kTm (PE), ACT exp with fused row-sum,
     DVE pad-correction + reciprocal, DVE scale, DMA out (stores alternate
     between the gpsimd and sync queues; HWDGE-only near the kernel end).

Schedule lessons baked in (see trace analysis in the session notes):
  - HAM clock gate: the PE runs at 1.2GHz until ~3.4us of CONTINUOUS busy;
    12 dummy warmup matmuls bridge from the framework preamble (~7.5us)
    to when the b0 inputs are consumable (~12.5us), and the stream stays
    dense after, so every real matmul runs at 2.4GHz.
  - Inputs ship as ONE big DMA per tensor (host pre-packs [P, dt*W+col]):
    data is consumable only once the issuing queue drains, so few big
    transfers beat many small ones.  Tiny tensors (biases, padc) lead
    their queue -- behind a 0.5MB weight DMA they'd land at ~21us and
    stall every epilogue.
  - The batch loop is software-pipelined one deep (proj(b+1) emitted
    before s_phase(b)) so proj epilogues precede the S softmax tail in
    the DVE/ACT queues -- otherwise the projection's 5th+ psum chains
    stall 1-2us at every batch boundary.
  - GpSimd cannot read PSUM, and its tensor ops run ~20x slower than DVE
    (Q7 DSP path) -- it only issues DMAs here.
"""

import sys

sys.path.insert(0, "/opt/trn_rl_repo")

import numpy as np
import ml_dtypes
from contextlib import ExitStack

import concourse.tile as tile
from concourse import bacc, mybir
from concourse.bass_utils import run_bass_kernel_spmd

P = 128
B, LQ, LK, D = 32, 1024, 1024, 512
NCORES = 8
BL = B // NCORES          # batches per core
NDT = D // P              # contraction tiles for projections
NET = D // P              # output-feature tiles (= S contraction tiles)
SCALE = float(1.0 / np.sqrt(D))
MASKC = -1.0e4

F32 = mybir.dt.float32
BF16 = mybir.dt.bfloat16
FP8 = mybir.dt.float8e4
AF = mybir.ActivationFunctionType

_CACHE = {}


def _chunks(width):
    """Split a free width into psum-bank-aligned chunks (<=512 each)."""
    out, c0 = [], 0
    while c0 < width:
        cw = min(512, width - c0)
        out.append((c0, cw))
        c0 += cw
    return out


def _body(tc, qT, kT, Wq, Wk, bq, bk, padc, maskc, out, NQP, NKP):
    nc = tc.nc
    # fp8 DoubleRow S-matmul measured L2 err 1.9e-2 vs the 2e-2 gate --
    # only ~3us faster than bf16 (S phase is ACT-bound), so keep bf16.
    NQB = (NQP + P - 1) // P  # S blocks per batch (last may be short)
    rows_of = lambda ib: min(P, NQP - ib * P)
    SPAD = ((NKP + 511) // 512) * 512   # psum tile width (bank aligned)
    kchunks = _chunks(NKP)
    qchunks = _chunks(NQP)
    use_mask = maskc is not None
    with ExitStack() as ctx:
        consts = ctx.enter_context(tc.tile_pool(name="consts", bufs=1))
        wpool = ctx.enter_context(tc.tile_pool(name="w", bufs=1))
        inpool = ctx.enter_context(tc.tile_pool(name="inp", bufs=2))
        actpool = ctx.enter_context(tc.tile_pool(name="act", bufs=2))
        mpool = ctx.enter_context(tc.tile_pool(name="mask", bufs=2))
        epool = ctx.enter_context(tc.tile_pool(name="exp", bufs=3))
        opool = ctx.enter_context(tc.tile_pool(name="pout", bufs=3))
        stpool = ctx.enter_context(tc.tile_pool(name="stat", bufs=6))
        ppsum = ctx.enter_context(tc.tile_pool(name="ppsum", bufs=4, space="PSUM"))
        spsum = ctx.enter_context(tc.tile_pool(name="spsum", bufs=2, space="PSUM"))

        # One big DMA per tensor (host pre-shuffles to the SBUF layout
        # [P, dt*W + col]): data consumability lags until the issuing QUEUE
        # works through its backlog (~0.7-1.3us per DMA regardless of
        # size), so 4 weight tiles as one 512KB DMA beat 4 x 128KB.
        wk_sb = wpool.tile([P, NDT * D], BF16, name="wk")
        wq_sb = wpool.tile([P, NDT * D], BF16, name="wq")
        nc.scalar.dma_start(out=wk_sb[:], in_=Wk[:])

        # bias tiles lead the gpsimd queue (tiny, ~2KB each): they must not
        # trail a big weight transfer, or the first relu epilogues (and with
        # them all psum recycling) stall until the whole queue drains
        bk_sb = consts.tile([P, NET], F32)
        bq_sb = consts.tile([P, NET], F32)
        nc.gpsimd.dma_start(out=bk_sb[:], in_=bk[:])
        nc.gpsimd.dma_start(out=bq_sb[:], in_=bq[:])

        # PE warmup: dummy matmuls on scratch tiles while the input DMAs are
        # in flight.  The HAM clock-gate needs ~3.4us of CONTINUOUS PE busy
        # before it lifts the 1.2GHz cold throttle; the trace with a 5-MM
        # warmup showed HAM firing only at t=21us (all of b0's projections
        # and S phase ran at half clock).  7 MMs bridge ~7.5->10.5us, by
        # which time Wk+xk (the k-proj critical set, ~1.1MB loaded first
        # across all three queues) have landed, so the k-proj stream keeps
        # the PE busy through the HAM window (~10.9us) and everything after
        # runs at 2.4GHz.  Results are never read.
        warm_in = consts.tile([P, 512], BF16, name="warm_in")
        nc.vector.memset(warm_in[:], 0.0)
        warm_ps = ppsum.tile([P, 512], F32, tag="proj", name="warm_ps")
        for _ in range(12):
            nc.tensor.matmul(
                warm_ps[:], lhsT=warm_in[:, 0:P], rhs=warm_in[:],
                start=True, stop=True,
            )

        def load_inputs(b):
            # One DMA per input tensor per batch: b0 queue depth is <=3
            # everywhere (scalar=[Wk,bk,bq], sync=[xk,Wq], gpsimd=[xq,padc])
            # so everything is consumable by ~10.5us.
            xk = inpool.tile([P, NDT * NKP], BF16, tag="xk")
            nc.sync.dma_start(out=xk[:], in_=kT[b])
            if b == 0:
                nc.sync.dma_start(out=wq_sb[:], in_=Wq[:])
            pad_sb = mpool.tile([P, 1], F32, tag="padc")
            nc.gpsimd.dma_start(out=pad_sb[:], in_=padc[b])
            xq = inpool.tile([P, NDT * NQP], BF16, tag="xq")
            nc.gpsimd.dma_start(out=xq[:], in_=qT[b])
            mask_sb = None
            if use_mask:
                mask_sb = mpool.tile([P, NKP], BF16, tag="maskc")
                nc.gpsimd.dma_start(out=mask_sb[:], in_=maskc[b])
            return xk, xq, pad_sb, mask_sb

        def relu_epilogue(ps, bias_sb, out_tiles, et, c0, cw):
            # The psum->SBUF relu copy is expensive on BOTH capable engines
            # for a 512-wide chunk (~740ns measured on ACT and DVE alike --
            # psum reads run ~1 elem/cycle/lane; GpSimd cannot touch PSUM
            # at all).  Split the 8 wide epilogues per batch 2+2 per proj
            # between DVE and ACT (the old all-on-ACT split made ACT a
            # co-bottleneck at 11us/batch); narrow (48-wide) ones are cheap
            # (~150ns) and go to DVE.
            if cw >= 256 and et % 2 == 1:
                nc.scalar.activation(
                    out=out_tiles[et][:, c0:c0 + cw],
                    in_=ps,
                    func=AF.Relu,
                    bias=bias_sb[:, et:et + 1],
                    scale=1.0,
                )
            else:
                # (psum + bias) max 0 -- exact relu+bias as one DVE op
                nc.vector.tensor_scalar(
                    out=out_tiles[et][:, c0:c0 + cw],
                    in0=ps,
                    scalar1=bias_sb[:, et:et + 1],
                    scalar2=0.0,
                    op0=mybir.AluOpType.add,
                    op1=mybir.AluOpType.max,
                )

        def proj(xin, xw, w_sb, bias_sb, out_tiles, chunks):
            # out_tiles[et] = relu(W[:, et].T @ x + b); xin is the packed
            # [P, NDT*xw] input tile, w_sb the packed [P, NDT*D] weights
            for et in range(NET):
                for (c0, cw) in chunks:
                    ps = ppsum.tile([P, 512], F32, tag="proj")
                    for dt_ in range(NDT):
                        nc.tensor.matmul(
                            ps[:, 0:cw],
                            lhsT=w_sb[:, dt_ * D + et * P:dt_ * D + (et + 1) * P],
                            rhs=xin[:, dt_ * xw + c0:dt_ * xw + c0 + cw],
                            start=(dt_ == 0),
                            stop=(dt_ == NDT - 1),
                        )
                    relu_epilogue(ps[:, 0:cw], bias_sb, out_tiles, et, c0, cw)

        def mask_add(kraw, mask_sb, b):
            kTm = [actpool.tile([P, NKP], BF16, tag=f"kTm{et}",
                                name=f"kTm{et}_{b}")
                   for et in range(NET)]
            for et in range(NET):
                # split across gpsimd and vector so neither gates the S phase
                eng = nc.gpsimd if et % 2 == 0 else nc.vector
                eng.tensor_add(kTm[et][:], kraw[et][:], mask_sb[:])
            return kTm

        def s_stats(rs, pad_sb, rows=P):
            # row-sum -> subtract pad-column contribution -> reciprocal
            # (all on DVE: a cross-engine sub->recip chain measurably
            # stalls DVE head-of-line behind GpSimd's store issues)
            rsv = stpool.tile([P, 1], F32, tag="rsv")
            nc.vector.tensor_tensor(
                out=rsv[0:rows, :], in0=rs[0:rows, :], in1=pad_sb[0:rows, :],
                op=mybir.AluOpType.subtract,
            )
            rc = stpool.tile([P, 1], F32, tag="recip")
            nc.vector.reciprocal(out=rc[0:rows, :], in_=rsv[0:rows, :])
            return rc

        def s_block(b, ib, qTt, kTm, pad_sb):
            rows = rows_of(ib)
            sp = spsum.tile([P, SPAD], F32, tag="S")
            for (c0, cw) in kchunks:
                for et in range(NET):
                    nc.tensor.matmul(
                        sp[0:rows, c0:c0 + cw],
                        lhsT=qTt[et][:, ib * P:ib * P + rows],
                        rhs=kTm[et][:, c0:c0 + cw],
                        start=(et == 0),
                        stop=(et == NET - 1),
                    )
            ex = epool.tile([P, NKP], BF16, tag="exp")
            rs = stpool.tile([P, 1], F32, tag="rowsum")
            nc.scalar.activation(
                out=ex[0:rows, :], in_=sp[0:rows, 0:NKP], func=AF.Exp,
                scale=SCALE, accum_out=rs[0:rows, :],
            )
            rc = s_stats(rs, pad_sb, rows)
            po = opool.tile([P, NKP], BF16, tag="po")
            # (GpSimd tensor ops measured ~20x slower than DVE -- Q7 DSP
            # path -- so this stays on DVE despite the queue pressure)
            nc.vector.tensor_scalar(
                out=po[0:rows, :], in0=ex[0:rows, :],
                scalar1=rc[0:rows, :], scalar2=None,
                op0=mybir.AluOpType.mult,
            )
            # alternate store queues so the output backlog drains 2x faster
            # (sync, not scalar: scalar's ACT must not stall behind DMA issue).
            # The last batch's late stores avoid gpsimd: its SWDGE path
            # completes ~2us after issue and the end-of-kernel queue DRAIN
            # would sit on the critical path.
            eng = nc.gpsimd if (ib % 2 == 0 and not
                                (b == BL - 1 and ib >= 2)) else nc.sync
            eng.dma_start(out=out[b, ib * P:ib * P + rows, :],
                          in_=po[0:rows, :])

        def s_block_final(b, ib, qTt, kTm, pad_sb, last=True):
            # Last block of the kernel: chunk-major matmuls into separate
            # 1-bank psums + a fully split epilogue so the first chunk's
            # exp/mul/store overlap the second chunk's matmuls and exp --
            # shortening the serial tail after the last MM.
            rows = rows_of(ib)
            nch = len(kchunks)
            # narrow chunk FIRST: its exp+accumulator-read run under the
            # wide chunk's matmuls, so the post-last-MM serial chain is just
            # exp(wide) -> RA -> stats -> scale -> store
            korder = list(enumerate(kchunks))[::-1] if last else \
                list(enumerate(kchunks))
            sps, rss, exs = [], [], []
            for ci, (c0, cw) in enumerate(kchunks):
                sps.append(ppsum.tile([P, 512], F32, tag="proj",
                                      name=f"fsp{ci}"))
                rss.append(stpool.tile([P, 1], F32, tag=f"rowsum{ci}",
                                       name=f"frs{ci}"))
                exs.append(epool.tile([P, cw], BF16, tag=f"fex{ci}",
                                      name=f"fex{ci}"))
            for ci, (c0, cw) in korder:
                for et in range(NET):
                    nc.tensor.matmul(
                        sps[ci][0:rows, 0:cw],
                        lhsT=qTt[et][:, ib * P:ib * P + rows],
                        rhs=kTm[et][:, c0:c0 + cw],
                        start=(et == 0),
                        stop=(et == NET - 1),
                    )
                nc.scalar.activation(
                    out=exs[ci][0:rows, :], in_=sps[ci][0:rows, 0:cw],
                    func=AF.Exp, scale=SCALE, accum_out=rss[ci][0:rows, :],
                )
            rs = rss[0]
            for ci in range(1, nch):
                rst = stpool.tile([P, 1], F32, tag="rowsumt", name=f"frt{ci}")
                nc.vector.tensor_tensor(
                    out=rst[0:rows, :], in0=rs[0:rows, :],
                    in1=rss[ci][0:rows, :],
                    op=mybir.AluOpType.add)
                rs = rst
            rc = s_stats(rs, pad_sb, rows)
            for ci, (c0, cw) in enumerate(kchunks):
                poh = opool.tile([P, cw], BF16, tag=f"fpo{ci}", name=f"fpo{ci}")
                nc.vector.tensor_scalar(
                    out=poh[0:rows, :], in0=exs[ci][0:rows, :],
                    scalar1=rc[0:rows, :], scalar2=None,
                    op0=mybir.AluOpType.mult,
                )
                if cw > 256:
                    # split the store across two queues so the final
                    # transfers drain 2x faster.  scalar only on the very
                    # last block -- earlier its queue still owes exps, and
                    # a ~650ns store issue there delays the final exp.
                    # HWDGE queues only (sync+scalar): gpsimd completion
                    # latency would stretch the final drain by ~2us.
                    h = cw // 2
                    eng2 = nc.scalar if last else nc.sync
                    nc.sync.dma_start(
                        out=out[b, ib * P:ib * P + rows, c0:c0 + h],
                        in_=poh[0:rows, 0:h])
                    eng2.dma_start(
                        out=out[b, ib * P:ib * P + rows, c0 + h:c0 + cw],
                        in_=poh[0:rows, h:cw])
                else:
                    eng = nc.scalar if last else nc.sync
                    eng.dma_start(
                        out=out[b, ib * P:ib * P + rows, c0:c0 + cw],
                        in_=poh[0:rows, :],
                    )

        def s_phase(b, qTt, kTm, pad_sb):
            for ib in range(NQB):
                if b == BL - 1 and ib >= NQB - 2:
                    # last two blocks: per-chunk psum + split exp, so the
                    # Scalar queue drains before the final serial epilogue
                    s_block_final(b, ib, qTt, kTm, pad_sb,
                                  last=(ib == NQB - 1))
                else:
                    s_block(b, ib, qTt, kTm, pad_sb)

        # Software-pipelined one batch deep: proj(b) is EMITTED before
        # s_phase(b-1), so proj(b)'s psum-draining epilogues sit in the
        # DVE/ACT queues AHEAD of S(b-1)'s softmax tail.  With the old
        # order, proj(b)'s 5th+ chains stalled ~1-2us at every batch
        # boundary waiting for an epilogue queued behind the S stats.
        cur = load_inputs(0)
        prev = None
        for b in range(BL):
            xk, xq, pad_sb, mask_sb = cur
            ktag = "kraw" if use_mask else "kTm"
            kraw = [actpool.tile([P, NKP], BF16, tag=f"{ktag}{et}",
                                 name=f"{ktag}{et}_{b}")
                    for et in range(NET)]
            proj(xk, NKP, wk_sb, bk_sb, kraw, kchunks)
            kTm = mask_add(kraw, mask_sb, b) if use_mask else kraw
            qTt = [actpool.tile([P, NQP], BF16, tag=f"qT{et}",
                                name=f"qT{et}_{b}")
                   for et in range(NET)]
            proj(xq, NQP, wq_sb, bq_sb, qTt, qchunks)
            if b + 1 < BL:
                cur = load_inputs(b + 1)
            if prev is not None:
                s_phase(b - 1, *prev)
            prev = (qTt, kTm, pad_sb)
        s_phase(BL - 1, *prev)


def _build(NQP, NKP, use_mask):
    nc = bacc.Bacc(
        "TRN2",
        target_bir_lowering=False,
        debug=False,
        enable_asserts=False,
        num_devices=NCORES,
    )
    qT = nc.dram_tensor("qT", [BL, P, NDT * NQP], BF16, kind="ExternalInput").ap()
    kT = nc.dram_tensor("kT", [BL, P, NDT * NKP], BF16, kind="ExternalInput").ap()
    Wq = nc.dram_tensor("Wq", [P, NDT * D], BF16, kind="ExternalInput").ap()
    Wk = nc.dram_tensor("Wk", [P, NDT * D], BF16, kind="ExternalInput").ap()
    bq = nc.dram_tensor("bq", [P, NET], F32, kind="ExternalInput").ap()
    bk = nc.dram_tensor("bk", [P, NET], F32, kind="ExternalInput").ap()
    padc = nc.dram_tensor("padc", [BL, P, 1], F32, kind="ExternalInput").ap()
    maskc = None
    if use_mask:
        maskc = nc.dram_tensor(
            "maskc", [BL, P, NKP], BF16, kind="ExternalInput").ap()
    out = nc.dram_tensor("out", [BL, NQP, NKP], BF16, kind="ExternalOutput").ap()

    with tile.TileContext(nc) as tc:
        _body(tc, qT, kT, Wq, Wk, bq, bk, padc, maskc, out, NQP, NKP)
    nc.compile()
    return nc


def _get_nc(NQP, NKP, use_mask):
    key = (NQP, NKP, use_mask)
    if key not in _CACHE:
        _CACHE[key] = _build(*key)
    return _CACHE[key]


def _pad16(n):
    # 16-col granularity: tail matmuls are free-dim-priced (no LDW floor),
    # so finer padding directly cuts PE cycles (576 -> 560 for this data,
    # ~3% of the matmul work).  S-blocks still span 128 rows; a short
    # trailing row-block costs the same per column.
    return max(64, ((n + 15) // 16) * 16)


def _prep(query, key, query_mask, key_mask, Wq, bq, Wk, bk):
    bf = ml_dtypes.bfloat16
    query = np.asarray(query, dtype=np.float32)
    key = np.asarray(key, dtype=np.float32)
    qmask = np.asarray(query_mask) != 0
    kmask = np.asarray(key_mask) != 0
    qidx = [np.nonzero(qmask[g])[0] for g in range(B)]
    kidx = [np.nonzero(kmask[g])[0] for g in range(B)]
    NQP = _pad16(max(len(i) for i in qidx))
    NKP = _pad16(max(len(i) for i in kidx))
    use_mask = bool(np.any(np.asarray(bk, dtype=np.float32) != 0.0))

    # device layout [P, dt*W + col]: row p of dt-block dt holds source row
    # dt*128+p -- lets the whole tensor ship as ONE contiguous DMA
    def pack(m):  # [D, W] -> [P, NDT*W]
        W = m.shape[1]
        return m.reshape(NDT, P, W).transpose(1, 0, 2).reshape(P, NDT * W)

    Wq_b = pack(np.asarray(Wq, dtype=np.float32).astype(bf))
    Wk_b = pack(np.asarray(Wk, dtype=np.float32).astype(bf))
    # bias for feature e lives at partition e%128, column e//128
    bq_t = np.asarray(bq, dtype=np.float32).reshape(NET, P).T.copy()
    bk_t = np.asarray(bk, dtype=np.float32).reshape(NET, P).T.copy()

    in_maps = []
    for c in range(NCORES):
        qTc = np.zeros((BL, P, NDT * NQP), dtype=bf)
        kTc = np.zeros((BL, P, NDT * NKP), dtype=bf)
        padc = np.zeros((BL, P, 1), dtype=np.float32)
        imap = {"qT": qTc, "kT": kTc, "Wq": Wq_b, "Wk": Wk_b,
                "bq": bq_t, "bk": bk_t, "padc": padc}
        if use_mask:
            mk = np.zeros((BL, P, NKP), dtype=bf)
            imap["maskc"] = mk
        for b in range(BL):
            g = c * BL + b
            qi, ki = qidx[g], kidx[g]
            qt = np.zeros((D, NQP), dtype=bf)
            kt = np.zeros((D, NKP), dtype=bf)
            qt[:, :len(qi)] = query[g][qi].T.astype(bf)
            kt[:, :len(ki)] = key[g][ki].T.astype(bf)
            qTc[b] = pack(qt)
            kTc[b] = pack(kt)
            if use_mask:
                imap["maskc"][b, :, len(ki):] = bf(MASKC)
            else:
                padc[b, :, 0] = float(NKP - len(ki))
        in_maps.append(imap)
    return in_maps, qidx, kidx, NQP, NKP, use_mask


def run(query, key, query_mask, key_mask, Wq, bq, Wk, bk, **kwargs):
    """Run on hardware; returns (output, BassKernelResults)."""
    in_maps, qidx, kidx, NQP, NKP, use_mask = _prep(
        query, key, query_mask, key_mask, Wq, bq, Wk, bk)
    nc = _get_nc(NQP, NKP, use_mask)
    res = run_bass_kernel_spmd(nc, in_maps, core_ids=list(range(NCORES)),
                               **kwargs)
    full = np.zeros((B, LQ, LK), dtype=np.float32)
    for c in range(NCORES):
        oc = res.results[c]["out"]
        for b in range(BL):
            g = c * BL + b
            qi, ki = qidx[g], kidx[g]
            full[g][np.ix_(qi, ki)] = oc[b][:len(qi), :len(ki)].astype(np.float32)
    return full, res


def kernel(query, key, query_mask, key_mask, Wq, bq, Wk, bk):
    full, _ = run(query, key, query_mask, key_mask, Wq, bq, Wk, bk)
    return full



# revision 56
# speedup vs baseline: 1.0459x; 1.0459x over previous
"""Masked attention-weight kernel (dense_transformer) for 8 TRN2 NeuronCores.

Computes, for inputs query/key [32,1024,512] f32, masks [32,1024] i32:
    q = relu(query @ Wq + bq); k = relu(key @ Wk + bk)
    w = softmax((q @ k^T)/sqrt(512) + key_mask_additive) * query_mask
Output: [32, 1024, 1024] f32.

Strategy: data-parallel over batch (4 batches/core, no collectives) PLUS
host-side mask compaction.  Masked key columns have weight exactly 0 in the
reference (exp(-1e9) underflows) and masked query rows are zeroed, so the
host gathers only the valid ~512 query rows / key columns per batch, pads
them to a fixed NQP/NKP (multiple of 64, 576 for this data), and the device
runs dense attention on the compacted [NQP, NKP] problem -- ~2.4x fewer
matmul cycles than the full [1024,1024].  The host scatters the compact
bf16 output back into a zero-filled full-size f32 array.

Padded key columns are all-zero inputs, so (with zero bias -- true for this
problem) their projected features are 0, their logits are 0, and each
contributes exp(0)=1 to the softmax row-sum; the device subtracts the
host-provided pad count from the row-sum before taking the reciprocal.
If the key bias were nonzero the host instead ships an additive -1e4
column mask applied to the projected k (use_mask variant).

Per-core pipeline, per batch (all matmuls bf16 with f32 PSUM):
  1. kTm[e,j] = relu(Wk.T @ keyT + bk): PE matmuls in (512,48) psum-bank
     chunks -> relu+bias epilogue (wide chunks alternate DVE/ACT, ~740ns
     each since psum reads are 1 elem/cycle/lane on both; narrow on DVE).
  2. qT[e,i] likewise.
  3. Per 128-row block: S = qT.T @ kTm (PE), ACT exp with fused row-sum,
     DVE pad-correction + reciprocal, DVE scale, DMA out (stores alternate
     between the gpsimd and sync queues; HWDGE-only near the kernel end).

Schedule lessons baked in (see trace analysis in the session notes):
  - HAM clock gate: the PE runs at 1.2GHz until ~3.4us of CONTINUOUS busy;
    12 dummy warmup matmuls bridge from the framework preamble (~7.5us)
    to when the b0 inputs are consumable (~12.5us), and the stream stays
    dense after, so every real matmul runs at 2.4GHz.
  - Inputs ship as ONE big DMA per tensor (host pre-packs [P, dt*W+col]):
    data is consumable only once the issuing queue drains, so few big
    transfers beat many small ones.  Tiny tensors (biases, padc) lead
    their queue -- behind a 0.5MB weight DMA they'd land at ~21us and
    stall every epilogue.
  - The batch loop is software-pipelined one deep (proj(b+1) emitted
    before s_phase(b)) so proj epilogues precede the S softmax tail in
    the DVE/ACT queues -- otherwise the projection's 5th+ psum chains
    stall 1-2us at every batch boundary.
  - GpSimd cannot read PSUM, and its tensor ops run ~20x slower than DVE
    (Q7 DSP path) -- it only issues DMAs here.
"""

import sys

sys.path.insert(0, "/opt/trn_rl_repo")

import numpy as np
import ml_dtypes
from contextlib import ExitStack

import concourse.tile as tile
from concourse import bacc, mybir
from concourse.bass_utils import run_bass_kernel_spmd

P = 128
B, LQ, LK, D = 32, 1024, 1024, 512
NCORES = 8
BL = B // NCORES          # batches per core
NDT = D // P              # contraction tiles for projections
NET = D // P              # output-feature tiles (= S contraction tiles)
SCALE = float(1.0 / np.sqrt(D))
MASKC = -1.0e4

F32 = mybir.dt.float32
BF16 = mybir.dt.bfloat16
FP8 = mybir.dt.float8e4
AF = mybir.ActivationFunctionType

_CACHE = {}


def _chunks(width):
    """Split a free width into psum-bank-aligned chunks (<=512 each)."""
    out, c0 = [], 0
    while c0 < width:
        cw = min(512, width - c0)
        out.append((c0, cw))
        c0 += cw
    return out


def _body(tc, qT, kT, Wq, Wk, bq, bk, padc, maskc, out, NQP, NKP):
    nc = tc.nc
    # fp8 DoubleRow S-matmul measured L2 err 1.9e-2 vs the 2e-2 gate --
    # only ~3us faster than bf16 (S phase is ACT-bound), so keep bf16.
    NQB = (NQP + P - 1) // P  # S blocks per batch (last may be short)
    rows_of = lambda ib: min(P, NQP - ib * P)
    SPAD = ((NKP + 511) // 512) * 512   # psum tile width (bank aligned)
    kchunks = _chunks(NKP)
    qchunks = _chunks(NQP)
    use_mask = maskc is not None
    with ExitStack() as ctx:
        consts = ctx.enter_context(tc.tile_pool(name="consts", bufs=1))
        wpool = ctx.enter_context(tc.tile_pool(name="w", bufs=1))
        inpool = ctx.enter_context(tc.tile_pool(name="inp", bufs=2))
        actpool = ctx.enter_context(tc.tile_pool(name="act", bufs=2))
        mpool = ctx.enter_context(tc.tile_pool(name="mask", bufs=2))
        epool = ctx.enter_context(tc.tile_pool(name="exp", bufs=3))
        opool = ctx.enter_context(tc.tile_pool(name="pout", bufs=3))
        stpool = ctx.enter_context(tc.tile_pool(name="stat", bufs=6))
        ppsum = ctx.enter_context(tc.tile_pool(name="ppsum", bufs=4, space="PSUM"))
        spsum = ctx.enter_context(tc.tile_pool(name="spsum", bufs=2, space="PSUM"))

        # One big DMA per tensor (host pre-shuffles to the SBUF layout
        # [P, dt*W + col]): data consumability lags until the issuing QUEUE
        # works through its backlog (~0.7-1.3us per DMA regardless of
        # size), so 4 weight tiles as one 512KB DMA beat 4 x 128KB.
        wk_sb = wpool.tile([P, NDT * D], BF16, name="wk")
        wq_sb = wpool.tile([P, NDT * D], BF16, name="wq")
        nc.scalar.dma_start(out=wk_sb[:], in_=Wk[:])

        # bias tiles lead the gpsimd queue (tiny, ~2KB each): they must not
        # trail a big weight transfer, or the first relu epilogues (and with
        # them all psum recycling) stall until the whole queue drains
        bk_sb = consts.tile([P, NET], F32)
        bq_sb = consts.tile([P, NET], F32)
        nc.gpsimd.dma_start(out=bk_sb[:], in_=bk[:])
        nc.gpsimd.dma_start(out=bq_sb[:], in_=bq[:])

        # PE warmup: dummy matmuls on scratch tiles while the input DMAs are
        # in flight.  The HAM clock-gate needs ~3.4us of CONTINUOUS PE busy
        # before it lifts the 1.2GHz cold throttle; the trace with a 5-MM
        # warmup showed HAM firing only at t=21us (all of b0's projections
        # and S phase ran at half clock).  7 MMs bridge ~7.5->10.5us, by
        # which time Wk+xk (the k-proj critical set, ~1.1MB loaded first
        # across all three queues) have landed, so the k-proj stream keeps
        # the PE busy through the HAM window (~10.9us) and everything after
        # runs at 2.4GHz.  Results are never read.
        warm_in = consts.tile([P, 512], BF16, name="warm_in")
        nc.vector.memset(warm_in[:], 0.0)
        warm_ps = ppsum.tile([P, 512], F32, tag="proj", name="warm_ps")
        for _ in range(12):
            nc.tensor.matmul(
                warm_ps[:], lhsT=warm_in[:, 0:P], rhs=warm_in[:],
                start=True, stop=True,
            )

        def load_inputs(b):
            # One DMA per input tensor per batch: b0 queue depth is <=3
            # everywhere (scalar=[Wk,bk,bq], sync=[xk,Wq], gpsimd=[xq,padc])
            # so everything is consumable by ~10.5us.
            xk = inpool.tile([P, NDT * NKP], BF16, tag="xk")
            nc.sync.dma_start(out=xk[:], in_=kT[b])
            if b == 0:
                nc.sync.dma_start(out=wq_sb[:], in_=Wq[:])
            pad_sb = mpool.tile([P, 1], F32, tag="padc")
            nc.gpsimd.dma_start(out=pad_sb[:], in_=padc[b])
            xq = inpool.tile([P, NDT * NQP], BF16, tag="xq")
            nc.gpsimd.dma_start(out=xq[:], in_=qT[b])
            mask_sb = None
            if use_mask:
                mask_sb = mpool.tile([P, NKP], BF16, tag="maskc")
                nc.gpsimd.dma_start(out=mask_sb[:], in_=maskc[b])
            return xk, xq, pad_sb, mask_sb

        def relu_epilogue(ps, bias_sb, out_tiles, et, c0, cw):
            # The psum->SBUF relu copy is expensive on BOTH capable engines
            # for a 512-wide chunk (~740ns measured on ACT and DVE alike --
            # psum reads run ~1 elem/cycle/lane; GpSimd cannot touch PSUM
            # at all).  Split the 8 wide epilogues per batch 2+2 per proj
            # between DVE and ACT (the old all-on-ACT split made ACT a
            # co-bottleneck at 11us/batch); narrow (48-wide) ones are cheap
            # (~150ns) and go to DVE.
            if cw >= 256 and et % 2 == 1:
                nc.scalar.activation(
                    out=out_tiles[et][:, c0:c0 + cw],
                    in_=ps,
                    func=AF.Relu,
                    bias=bias_sb[:, et:et + 1],
                    scale=1.0,
                )
            else:
                # (psum + bias) max 0 -- exact relu+bias as one DVE op
                nc.vector.tensor_scalar(
                    out=out_tiles[et][:, c0:c0 + cw],
                    in0=ps,
                    scalar1=bias_sb[:, et:et + 1],
                    scalar2=0.0,
                    op0=mybir.AluOpType.add,
                    op1=mybir.AluOpType.max,
                )

        def proj(xin, xw, w_sb, bias_sb, out_tiles, chunks):
            # out_tiles[et] = relu(W[:, et].T @ x + b); xin is the packed
            # [P, NDT*xw] input tile, w_sb the packed [P, NDT*D] weights
            for et in range(NET):
                for (c0, cw) in chunks:
                    ps = ppsum.tile([P, 512], F32, tag="proj")
                    for dt_ in range(NDT):
                        nc.tensor.matmul(
                            ps[:, 0:cw],
                            lhsT=w_sb[:, dt_ * D + et * P:dt_ * D + (et + 1) * P],
                            rhs=xin[:, dt_ * xw + c0:dt_ * xw + c0 + cw],
                            start=(dt_ == 0),
                            stop=(dt_ == NDT - 1),
                        )
                    relu_epilogue(ps[:, 0:cw], bias_sb, out_tiles, et, c0, cw)

        def mask_add(kraw, mask_sb, b):
            kTm = [actpool.tile([P, NKP], BF16, tag=f"kTm{et}",
                                name=f"kTm{et}_{b}")
                   for et in range(NET)]
            for et in range(NET):
                # split across gpsimd and vector so neither gates the S phase
                eng = nc.gpsimd if et % 2 == 0 else nc.vector
                eng.tensor_add(kTm[et][:], kraw[et][:], mask_sb[:])
            return kTm

        def s_stats(rs, pad_sb, rows=P):
            # row-sum -> subtract pad-column contribution -> reciprocal
            # (all on DVE: a cross-engine sub->recip chain measurably
            # stalls DVE head-of-line behind GpSimd's store issues)
            rsv = stpool.tile([P, 1], F32, tag="rsv")
            nc.vector.tensor_tensor(
                out=rsv[0:rows, :], in0=rs[0:rows, :], in1=pad_sb[0:rows, :],
                op=mybir.AluOpType.subtract,
            )
            rc = stpool.tile([P, 1], F32, tag="recip")
            nc.vector.reciprocal(out=rc[0:rows, :], in_=rsv[0:rows, :])
            return rc

        def s_block(b, ib, qTt, kTm, pad_sb):
            rows = rows_of(ib)
            sp = spsum.tile([P, SPAD], F32, tag="S")
            for (c0, cw) in kchunks:
                for et in range(NET):
                    nc.tensor.matmul(
                        sp[0:rows, c0:c0 + cw],
                        lhsT=qTt[et][:, ib * P:ib * P + rows],
                        rhs=kTm[et][:, c0:c0 + cw],
                        start=(et == 0),
                        stop=(et == NET - 1),
                    )
            ex = epool.tile([P, NKP], BF16, tag="exp")
            rs = stpool.tile([P, 1], F32, tag="rowsum")
            nc.scalar.activation(
                out=ex[0:rows, :], in_=sp[0:rows, 0:NKP], func=AF.Exp,
                scale=SCALE, accum_out=rs[0:rows, :],
            )
            rc = s_stats(rs, pad_sb, rows)
            po = opool.tile([P, NKP], BF16, tag="po")
            # (GpSimd tensor ops measured ~20x slower than DVE -- Q7 DSP
            # path -- so this stays on DVE despite the queue pressure)
            nc.vector.tensor_scalar(
                out=po[0:rows, :], in0=ex[0:rows, :],
                scalar1=rc[0:rows, :], scalar2=None,
                op0=mybir.AluOpType.mult,
            )
            # alternate store queues so the output backlog drains 2x faster
            # (sync, not scalar: scalar's ACT must not stall behind DMA issue).
            # The last batch's late stores avoid gpsimd: its SWDGE path
            # completes ~2us after issue and the end-of-kernel queue DRAIN
            # would sit on the critical path.
            eng = nc.gpsimd if (ib % 2 == 0 and not
                                (b == BL - 1 and ib >= 2)) else nc.sync
            eng.dma_start(out=out[b, ib * P:ib * P + rows, :],
                          in_=po[0:rows, :])

        def s_block_final(b, ib, qTt, kTm, pad_sb, last=True):
            # Last block of the kernel: chunk-major matmuls into separate
            # 1-bank psums + a fully split epilogue so the first chunk's
            # exp/mul/store overlap the second chunk's matmuls and exp --
            # shortening the serial tail after the last MM.
            rows = rows_of(ib)
            nch = len(kchunks)
            # narrow chunk FIRST: its exp+accumulator-read run under the
            # wide chunk's matmuls, so the post-last-MM serial chain is just
            # exp(wide) -> RA -> stats -> scale -> store
            korder = list(enumerate(kchunks))[::-1] if last else \
                list(enumerate(kchunks))
            sps, rss, exs = [], [], []
            for ci, (c0, cw) in enumerate(kchunks):
                sps.append(ppsum.tile([P, 512], F32, tag="proj",
                                      name=f"fsp{ci}"))
                rss.append(stpool.tile([P, 1], F32, tag=f"rowsum{ci}",
                                       name=f"frs{ci}"))
                exs.append(epool.tile([P, cw], BF16, tag=f"fex{ci}",
                                      name=f"fex{ci}"))
            for ci, (c0, cw) in korder:
                for et in range(NET):
                    nc.tensor.matmul(
                        sps[ci][0:rows, 0:cw],
                        lhsT=qTt[et][:, ib * P:ib * P + rows],
                        rhs=kTm[et][:, c0:c0 + cw],
                        start=(et == 0),
                        stop=(et == NET - 1),
                    )
                nc.scalar.activation(
                    out=exs[ci][0:rows, :], in_=sps[ci][0:rows, 0:cw],
                    func=AF.Exp, scale=SCALE, accum_out=rss[ci][0:rows, :],
                )
            rs = rss[0]
            for ci in range(1, nch):
                rst = stpool.tile([P, 1], F32, tag="rowsumt", name=f"frt{ci}")
                nc.vector.tensor_tensor(
                    out=rst[0:rows, :], in0=rs[0:rows, :],
                    in1=rss[ci][0:rows, :],
                    op=mybir.AluOpType.add)
                rs = rst
            rc = s_stats(rs, pad_sb, rows)
            for ci, (c0, cw) in enumerate(kchunks):
                poh = opool.tile([P, cw], BF16, tag=f"fpo{ci}", name=f"fpo{ci}")
                nc.vector.tensor_scalar(
                    out=poh[0:rows, :], in0=exs[ci][0:rows, :],
                    scalar1=rc[0:rows, :], scalar2=None,
                    op0=mybir.AluOpType.mult,
                )
                if cw > 256:
                    # split the store across two queues so the final
                    # transfers drain 2x faster.  scalar only on the very
                    # last block -- earlier its queue still owes exps, and
                    # a ~650ns store issue there delays the final exp.
                    # HWDGE queues only (sync+scalar): gpsimd completion
                    # latency would stretch the final drain by ~2us.
                    h = cw // 2
                    eng2 = nc.scalar if last else nc.sync
                    nc.sync.dma_start(
                        out=out[b, ib * P:ib * P + rows, c0:c0 + h],
                        in_=poh[0:rows, 0:h])
                    eng2.dma_start(
                        out=out[b, ib * P:ib * P + rows, c0 + h:c0 + cw],
                        in_=poh[0:rows, h:cw])
                else:
                    eng = nc.scalar if last else nc.sync
                    eng.dma_start(
                        out=out[b, ib * P:ib * P + rows, c0:c0 + cw],
                        in_=poh[0:rows, :],
                    )

        def s_phase(b, qTt, kTm, pad_sb):
            for ib in range(NQB):
                if b == BL - 1 and ib >= NQB - 2:
                    # last two blocks: per-chunk psum + split exp, so the
                    # Scalar queue drains before the final serial epilogue
                    s_block_final(b, ib, qTt, kTm, pad_sb,
                                  last=(ib == NQB - 1))
                else:
                    s_block(b, ib, qTt, kTm, pad_sb)

        # Software-pipelined one batch deep: proj(b) is EMITTED before
        # s_phase(b-1), so proj(b)'s psum-draining epilogues sit in the
        # DVE/ACT queues AHEAD of S(b-1)'s softmax tail.  With the old
        # order, proj(b)'s 5th+ chains stalled ~1-2us at every batch
        # boundary waiting for an epilogue queued behind the S stats.
        cur = load_inputs(0)
        prev = None
        for b in range(BL):
            xk, xq, pad_sb, mask_sb = cur
            ktag = "kraw" if use_mask else "kTm"
            kraw = [actpool.tile([P, NKP], BF16, tag=f"{ktag}{et}",
                                 name=f"{ktag}{et}_{b}")
                    for et in range(NET)]
            proj(xk, NKP, wk_sb, bk_sb, kraw, kchunks)
            if b == 0:
                # keep-warm fillers: xq/Wq land 0-2us after the k-proj's
                # matmuls run out (input-bandwidth bound); an idle PE here
                # risks a HAM re-throttle (observed in ~half the runs,
                # costing ~2-3us of half-clock matmuls).  Six dummy MMs
                # bridge ~1.3us of that window.
                wp2 = ppsum.tile([P, 512], F32, tag="proj", name="warm2")
                for _ in range(6):
                    nc.tensor.matmul(
                        wp2[:], lhsT=warm_in[:, 0:P], rhs=warm_in[:],
                        start=True, stop=True,
                    )
            kTm = mask_add(kraw, mask_sb, b) if use_mask else kraw
            qTt = [actpool.tile([P, NQP], BF16, tag=f"qT{et}",
                                name=f"qT{et}_{b}")
                   for et in range(NET)]
            proj(xq, NQP, wq_sb, bq_sb, qTt, qchunks)
            if b + 1 < BL:
                cur = load_inputs(b + 1)
            if prev is not None:
                s_phase(b - 1, *prev)
            prev = (qTt, kTm, pad_sb)
        s_phase(BL - 1, *prev)


def _build(NQP, NKP, use_mask):
    nc = bacc.Bacc(
        "TRN2",
        target_bir_lowering=False,
        debug=False,
        enable_asserts=False,
        num_devices=NCORES,
    )
    qT = nc.dram_tensor("qT", [BL, P, NDT * NQP], BF16, kind="ExternalInput").ap()
    kT = nc.dram_tensor("kT", [BL, P, NDT * NKP], BF16, kind="ExternalInput").ap()
    Wq = nc.dram_tensor("Wq", [P, NDT * D], BF16, kind="ExternalInput").ap()
    Wk = nc.dram_tensor("Wk", [P, NDT * D], BF16, kind="ExternalInput").ap()
    bq = nc.dram_tensor("bq", [P, NET], F32, kind="ExternalInput").ap()
    bk = nc.dram_tensor("bk", [P, NET], F32, kind="ExternalInput").ap()
    padc = nc.dram_tensor("padc", [BL, P, 1], F32, kind="ExternalInput").ap()
    maskc = None
    if use_mask:
        maskc = nc.dram_tensor(
            "maskc", [BL, P, NKP], BF16, kind="ExternalInput").ap()
    out = nc.dram_tensor("out", [BL, NQP, NKP], BF16, kind="ExternalOutput").ap()

    with tile.TileContext(nc) as tc:
        _body(tc, qT, kT, Wq, Wk, bq, bk, padc, maskc, out, NQP, NKP)
    nc.compile()
    return nc


def _get_nc(NQP, NKP, use_mask):
    key = (NQP, NKP, use_mask)
    if key not in _CACHE:
        _CACHE[key] = _build(*key)
    return _CACHE[key]


def _pad16(n):
    # 16-col granularity: tail matmuls are free-dim-priced (no LDW floor),
    # so finer padding directly cuts PE cycles (576 -> 560 for this data,
    # ~3% of the matmul work).  S-blocks still span 128 rows; a short
    # trailing row-block costs the same per column.
    return max(64, ((n + 15) // 16) * 16)


def _prep(query, key, query_mask, key_mask, Wq, bq, Wk, bk):
    bf = ml_dtypes.bfloat16
    query = np.asarray(query, dtype=np.float32)
    key = np.asarray(key, dtype=np.float32)
    qmask = np.asarray(query_mask) != 0
    kmask = np.asarray(key_mask) != 0
    qidx = [np.nonzero(qmask[g])[0] for g in range(B)]
    kidx = [np.nonzero(kmask[g])[0] for g in range(B)]
    NQP = _pad16(max(len(i) for i in qidx))
    NKP = _pad16(max(len(i) for i in kidx))
    use_mask = bool(np.any(np.asarray(bk, dtype=np.float32) != 0.0))

    # device layout [P, dt*W + col]: row p of dt-block dt holds source row
    # dt*128+p -- lets the whole tensor ship as ONE contiguous DMA
    def pack(m):  # [D, W] -> [P, NDT*W]
        W = m.shape[1]
        return m.reshape(NDT, P, W).transpose(1, 0, 2).reshape(P, NDT * W)

    Wq_b = pack(np.asarray(Wq, dtype=np.float32).astype(bf))
    Wk_b = pack(np.asarray(Wk, dtype=np.float32).astype(bf))
    # bias for feature e lives at partition e%128, column e//128
    bq_t = np.asarray(bq, dtype=np.float32).reshape(NET, P).T.copy()
    bk_t = np.asarray(bk, dtype=np.float32).reshape(NET, P).T.copy()

    in_maps = []
    for c in range(NCORES):
        qTc = np.zeros((BL, P, NDT * NQP), dtype=bf)
        kTc = np.zeros((BL, P, NDT * NKP), dtype=bf)
        padc = np.zeros((BL, P, 1), dtype=np.float32)
        imap = {"qT": qTc, "kT": kTc, "Wq": Wq_b, "Wk": Wk_b,
                "bq": bq_t, "bk": bk_t, "padc": padc}
        if use_mask:
            mk = np.zeros((BL, P, NKP), dtype=bf)
            imap["maskc"] = mk
        for b in range(BL):
            g = c * BL + b
            qi, ki = qidx[g], kidx[g]
            qt = np.zeros((D, NQP), dtype=bf)
            kt = np.zeros((D, NKP), dtype=bf)
            qt[:, :len(qi)] = query[g][qi].T.astype(bf)
            kt[:, :len(ki)] = key[g][ki].T.astype(bf)
            qTc[b] = pack(qt)
            kTc[b] = pack(kt)
            if use_mask:
                imap["maskc"][b, :, len(ki):] = bf(MASKC)
            else:
                padc[b, :, 0] = float(NKP - len(ki))
        in_maps.append(imap)
    return in_maps, qidx, kidx, NQP, NKP, use_mask


def run(query, key, query_mask, key_mask, Wq, bq, Wk, bk, **kwargs):
    """Run on hardware; returns (output, BassKernelResults)."""
    in_maps, qidx, kidx, NQP, NKP, use_mask = _prep(
        query, key, query_mask, key_mask, Wq, bq, Wk, bk)
    nc = _get_nc(NQP, NKP, use_mask)
    res = run_bass_kernel_spmd(nc, in_maps, core_ids=list(range(NCORES)),
                               **kwargs)
    full = np.zeros((B, LQ, LK), dtype=np.float32)
    for c in range(NCORES):
        oc = res.results[c]["out"]
        for b in range(BL):
            g = c * BL + b
            qi, ki = qidx[g], kidx[g]
            full[g][np.ix_(qi, ki)] = oc[b][:len(qi), :len(ki)].astype(np.float32)
    return full, res


def kernel(query, key, query_mask, key_mask, Wq, bq, Wk, bk):
    full, _ = run(query, key, query_mask, key_mask, Wq, bq, Wk, bk)
    return full



# revision 57
# speedup vs baseline: 1.0531x; 1.0068x over previous
"""Masked attention-weight kernel (dense_transformer) for 8 TRN2 NeuronCores.

Computes, for inputs query/key [32,1024,512] f32, masks [32,1024] i32:
    q = relu(query @ Wq + bq); k = relu(key @ Wk + bk)
    w = softmax((q @ k^T)/sqrt(512) + key_mask_additive) * query_mask
Output: [32, 1024, 1024] f32.

Strategy: data-parallel over batch (4 batches/core, no collectives) PLUS
host-side mask compaction.  Masked key columns have weight exactly 0 in the
reference (exp(-1e9) underflows) and masked query rows are zeroed, so the
host gathers only the valid ~512 query rows / key columns per batch, pads
them to a fixed NQP/NKP (multiple of 64, 576 for this data), and the device
runs dense attention on the compacted [NQP, NKP] problem -- ~2.4x fewer
matmul cycles than the full [1024,1024].  The host scatters the compact
bf16 output back into a zero-filled full-size f32 array.

Padded key columns are all-zero inputs, so (with zero bias -- true for this
problem) their projected features are 0, their logits are 0, and each
contributes exp(0)=1 to the softmax row-sum; the device subtracts the
host-provided pad count from the row-sum before taking the reciprocal.
If the key bias were nonzero the host instead ships an additive -1e4
column mask applied to the projected k (use_mask variant).

Per-core pipeline, per batch (all matmuls bf16 with f32 PSUM):
  1. kTm[e,j] = relu(Wk.T @ keyT + bk): PE matmuls in (512,48) psum-bank
     chunks -> relu+bias epilogue (wide chunks alternate DVE/ACT, ~740ns
     each since psum reads are 1 elem/cycle/lane on both; narrow on DVE).
  2. qT[e,i] likewise.
  3. Per 128-row block: S = qT.T @ kTm (PE), ACT exp with fused row-sum,
     DVE pad-correction + reciprocal, DVE scale, DMA out (stores alternate
     between the gpsimd and sync queues; HWDGE-only near the kernel end).

Schedule lessons baked in (see trace analysis in the session notes):
  - HAM clock gate: the PE runs at 1.2GHz until ~3.4us of CONTINUOUS busy;
    12 dummy warmup matmuls bridge from the framework preamble (~7.5us)
    to when the b0 inputs are consumable (~12.5us), and the stream stays
    dense after, so every real matmul runs at 2.4GHz.
  - Inputs ship as ONE big DMA per tensor (host pre-packs [P, dt*W+col]):
    data is consumable only once the issuing queue drains, so few big
    transfers beat many small ones.  Tiny tensors (biases, padc) lead
    their queue -- behind a 0.5MB weight DMA they'd land at ~21us and
    stall every epilogue.
  - The batch loop is software-pipelined one deep (proj(b+1) emitted
    before s_phase(b)) so proj epilogues precede the S softmax tail in
    the DVE/ACT queues -- otherwise the projection's 5th+ psum chains
    stall 1-2us at every batch boundary.
  - GpSimd cannot read PSUM, and its tensor ops run ~20x slower than DVE
    (Q7 DSP path) -- it only issues DMAs here.
"""

import sys

sys.path.insert(0, "/opt/trn_rl_repo")

import numpy as np
import ml_dtypes
from contextlib import ExitStack

import concourse.tile as tile
from concourse import bacc, mybir
from concourse.bass_utils import run_bass_kernel_spmd

P = 128
B, LQ, LK, D = 32, 1024, 1024, 512
NCORES = 8
BL = B // NCORES          # batches per core
NDT = D // P              # contraction tiles for projections
NET = D // P              # output-feature tiles (= S contraction tiles)
SCALE = float(1.0 / np.sqrt(D))
MASKC = -1.0e4

F32 = mybir.dt.float32
BF16 = mybir.dt.bfloat16
FP8 = mybir.dt.float8e4
AF = mybir.ActivationFunctionType

_CACHE = {}


def _chunks(width):
    """Split a free width into psum-bank-aligned chunks (<=512 each)."""
    out, c0 = [], 0
    while c0 < width:
        cw = min(512, width - c0)
        out.append((c0, cw))
        c0 += cw
    return out


def _body(tc, qT, kT, Wq, Wk, bq, bk, padc, maskc, out, NQP, NKP):
    nc = tc.nc
    # fp8 DoubleRow S-matmul measured L2 err 1.9e-2 vs the 2e-2 gate --
    # only ~3us faster than bf16 (S phase is ACT-bound), so keep bf16.
    NQB = (NQP + P - 1) // P  # S blocks per batch (last may be short)
    rows_of = lambda ib: min(P, NQP - ib * P)
    SPAD = ((NKP + 511) // 512) * 512   # psum tile width (bank aligned)
    kchunks = _chunks(NKP)
    qchunks = _chunks(NQP)
    use_mask = maskc is not None
    with ExitStack() as ctx:
        consts = ctx.enter_context(tc.tile_pool(name="consts", bufs=1))
        wpool = ctx.enter_context(tc.tile_pool(name="w", bufs=1))
        inpool = ctx.enter_context(tc.tile_pool(name="inp", bufs=2))
        actpool = ctx.enter_context(tc.tile_pool(name="act", bufs=2))
        mpool = ctx.enter_context(tc.tile_pool(name="mask", bufs=2))
        epool = ctx.enter_context(tc.tile_pool(name="exp", bufs=3))
        opool = ctx.enter_context(tc.tile_pool(name="pout", bufs=3))
        stpool = ctx.enter_context(tc.tile_pool(name="stat", bufs=6))
        ppsum = ctx.enter_context(tc.tile_pool(name="ppsum", bufs=4, space="PSUM"))
        spsum = ctx.enter_context(tc.tile_pool(name="spsum", bufs=2, space="PSUM"))

        # One big DMA per tensor (host pre-shuffles to the SBUF layout
        # [P, dt*W + col]): data consumability lags until the issuing QUEUE
        # works through its backlog (~0.7-1.3us per DMA regardless of
        # size), so 4 weight tiles as one 512KB DMA beat 4 x 128KB.
        wk_sb = wpool.tile([P, NDT * D], BF16, name="wk")
        wq_sb = wpool.tile([P, NDT * D], BF16, name="wq")
        nc.scalar.dma_start(out=wk_sb[:], in_=Wk[:])

        # bias tiles lead the gpsimd queue (tiny, ~2KB each): they must not
        # trail a big weight transfer, or the first relu epilogues (and with
        # them all psum recycling) stall until the whole queue drains
        bk_sb = consts.tile([P, NET], F32)
        bq_sb = consts.tile([P, NET], F32)
        nc.gpsimd.dma_start(out=bk_sb[:], in_=bk[:])
        nc.gpsimd.dma_start(out=bq_sb[:], in_=bq[:])

        # PE warmup: dummy matmuls on scratch tiles while the input DMAs are
        # in flight.  The HAM clock-gate needs ~3.4us of CONTINUOUS PE busy
        # before it lifts the 1.2GHz cold throttle; the trace with a 5-MM
        # warmup showed HAM firing only at t=21us (all of b0's projections
        # and S phase ran at half clock).  7 MMs bridge ~7.5->10.5us, by
        # which time Wk+xk (the k-proj critical set, ~1.1MB loaded first
        # across all three queues) have landed, so the k-proj stream keeps
        # the PE busy through the HAM window (~10.9us) and everything after
        # runs at 2.4GHz.  Results are never read.
        warm_in = consts.tile([P, 512], BF16, name="warm_in")
        nc.vector.memset(warm_in[:], 0.0)
        warm_ps = ppsum.tile([P, 512], F32, tag="proj", name="warm_ps")
        for _ in range(12):
            nc.tensor.matmul(
                warm_ps[:], lhsT=warm_in[:, 0:P], rhs=warm_in[:],
                start=True, stop=True,
            )

        def load_inputs(b):
            # One DMA per input tensor per batch: b0 queue depth is <=3
            # everywhere (scalar=[Wk,bk,bq], sync=[xk,Wq], gpsimd=[xq,padc])
            # so everything is consumable by ~10.5us.
            xk = inpool.tile([P, NDT * NKP], BF16, tag="xk")
            nc.sync.dma_start(out=xk[:], in_=kT[b])
            if b == 0:
                nc.sync.dma_start(out=wq_sb[:], in_=Wq[:])
            pad_sb = mpool.tile([P, 1], F32, tag="padc")
            nc.gpsimd.dma_start(out=pad_sb[:], in_=padc[b])
            xq = inpool.tile([P, NDT * NQP], BF16, tag="xq")
            nc.gpsimd.dma_start(out=xq[:], in_=qT[b])
            mask_sb = None
            if use_mask:
                mask_sb = mpool.tile([P, NKP], BF16, tag="maskc")
                nc.gpsimd.dma_start(out=mask_sb[:], in_=maskc[b])
            return xk, xq, pad_sb, mask_sb

        def relu_epilogue(ps, bias_sb, out_tiles, et, c0, cw):
            # The psum->SBUF relu copy is expensive on BOTH capable engines
            # for a 512-wide chunk (~740ns measured on ACT and DVE alike --
            # psum reads run ~1 elem/cycle/lane; GpSimd cannot touch PSUM
            # at all).  Split the 8 wide epilogues per batch 2+2 per proj
            # between DVE and ACT (the old all-on-ACT split made ACT a
            # co-bottleneck at 11us/batch); narrow (48-wide) ones are cheap
            # (~150ns) and go to DVE.
            if cw >= 256 and et % 2 == 1:
                nc.scalar.activation(
                    out=out_tiles[et][:, c0:c0 + cw],
                    in_=ps,
                    func=AF.Relu,
                    bias=bias_sb[:, et:et + 1],
                    scale=1.0,
                )
            else:
                # (psum + bias) max 0 -- exact relu+bias as one DVE op
                nc.vector.tensor_scalar(
                    out=out_tiles[et][:, c0:c0 + cw],
                    in0=ps,
                    scalar1=bias_sb[:, et:et + 1],
                    scalar2=0.0,
                    op0=mybir.AluOpType.add,
                    op1=mybir.AluOpType.max,
                )

        def proj(xin, xw, w_sb, bias_sb, out_tiles, chunks):
            # out_tiles[et] = relu(W[:, et].T @ x + b); xin is the packed
            # [P, NDT*xw] input tile, w_sb the packed [P, NDT*D] weights
            for et in range(NET):
                for (c0, cw) in chunks:
                    ps = ppsum.tile([P, 512], F32, tag="proj")
                    for dt_ in range(NDT):
                        nc.tensor.matmul(
                            ps[:, 0:cw],
                            lhsT=w_sb[:, dt_ * D + et * P:dt_ * D + (et + 1) * P],
                            rhs=xin[:, dt_ * xw + c0:dt_ * xw + c0 + cw],
                            start=(dt_ == 0),
                            stop=(dt_ == NDT - 1),
                        )
                    relu_epilogue(ps[:, 0:cw], bias_sb, out_tiles, et, c0, cw)

        def mask_add(kraw, mask_sb, b):
            kTm = [actpool.tile([P, NKP], BF16, tag=f"kTm{et}",
                                name=f"kTm{et}_{b}")
                   for et in range(NET)]
            for et in range(NET):
                # split across gpsimd and vector so neither gates the S phase
                eng = nc.gpsimd if et % 2 == 0 else nc.vector
                eng.tensor_add(kTm[et][:], kraw[et][:], mask_sb[:])
            return kTm

        def s_stats(rs, pad_sb, rows=P):
            # row-sum -> subtract pad-column contribution -> reciprocal
            # (all on DVE: a cross-engine sub->recip chain measurably
            # stalls DVE head-of-line behind GpSimd's store issues)
            rsv = stpool.tile([P, 1], F32, tag="rsv")
            nc.vector.tensor_tensor(
                out=rsv[0:rows, :], in0=rs[0:rows, :], in1=pad_sb[0:rows, :],
                op=mybir.AluOpType.subtract,
            )
            rc = stpool.tile([P, 1], F32, tag="recip")
            nc.vector.reciprocal(out=rc[0:rows, :], in_=rsv[0:rows, :])
            return rc

        def s_block(b, ib, qTt, kTm, pad_sb):
            rows = rows_of(ib)
            sp = spsum.tile([P, SPAD], F32, tag="S")
            for (c0, cw) in kchunks:
                for et in range(NET):
                    nc.tensor.matmul(
                        sp[0:rows, c0:c0 + cw],
                        lhsT=qTt[et][:, ib * P:ib * P + rows],
                        rhs=kTm[et][:, c0:c0 + cw],
                        start=(et == 0),
                        stop=(et == NET - 1),
                    )
            ex = epool.tile([P, NKP], BF16, tag="exp")
            rs = stpool.tile([P, 1], F32, tag="rowsum")
            nc.scalar.activation(
                out=ex[0:rows, :], in_=sp[0:rows, 0:NKP], func=AF.Exp,
                scale=SCALE, accum_out=rs[0:rows, :],
            )
            rc = s_stats(rs, pad_sb, rows)
            po = opool.tile([P, NKP], BF16, tag="po")
            # (GpSimd tensor ops measured ~20x slower than DVE -- Q7 DSP
            # path -- so this stays on DVE despite the queue pressure)
            nc.vector.tensor_scalar(
                out=po[0:rows, :], in0=ex[0:rows, :],
                scalar1=rc[0:rows, :], scalar2=None,
                op0=mybir.AluOpType.mult,
            )
            # alternate store queues so the output backlog drains 2x faster
            # (sync, not scalar: scalar's ACT must not stall behind DMA issue).
            # The last batch's late stores avoid gpsimd: its SWDGE path
            # completes ~2us after issue and the end-of-kernel queue DRAIN
            # would sit on the critical path.
            eng = nc.gpsimd if (ib % 2 == 0 and not
                                (b == BL - 1 and ib >= 2)) else nc.sync
            eng.dma_start(out=out[b, ib * P:ib * P + rows, :],
                          in_=po[0:rows, :])

        def s_block_final(b, ib, qTt, kTm, pad_sb):
            # Very last block of the kernel: chunk-major matmuls into
            # separate 1-bank psums, NARROW chunk first so its exp and
            # accumulator-read run under the wide chunk's matmuls -- the
            # post-last-MM serial chain is just exp(wide) -> RA -> stats ->
            # scale -> ONE store (the block is <=64 rows, ~55KB).
            rows = rows_of(ib)
            nch = len(kchunks)
            sps, rss, exs = [], [], []
            for ci, (c0, cw) in enumerate(kchunks):
                sps.append(ppsum.tile([P, 512], F32, tag="proj",
                                      name=f"fsp{ci}"))
                rss.append(stpool.tile([P, 1], F32, tag=f"rowsum{ci}",
                                       name=f"frs{ci}"))
                exs.append(epool.tile([P, cw], BF16, tag=f"fex{ci}",
                                      name=f"fex{ci}"))
            for ci, (c0, cw) in list(enumerate(kchunks))[::-1]:
                for et in range(NET):
                    nc.tensor.matmul(
                        sps[ci][0:rows, 0:cw],
                        lhsT=qTt[et][:, ib * P:ib * P + rows],
                        rhs=kTm[et][:, c0:c0 + cw],
                        start=(et == 0),
                        stop=(et == NET - 1),
                    )
                nc.scalar.activation(
                    out=exs[ci][0:rows, :], in_=sps[ci][0:rows, 0:cw],
                    func=AF.Exp, scale=SCALE, accum_out=rss[ci][0:rows, :],
                )
            rs = rss[0]
            for ci in range(1, nch):
                rst = stpool.tile([P, 1], F32, tag="rowsumt", name=f"frt{ci}")
                nc.vector.tensor_tensor(
                    out=rst[0:rows, :], in0=rs[0:rows, :],
                    in1=rss[ci][0:rows, :],
                    op=mybir.AluOpType.add)
                rs = rst
            rc = s_stats(rs, pad_sb, rows)
            po = opool.tile([P, NKP], BF16, tag="po", name="fpo")
            for ci, (c0, cw) in enumerate(kchunks):
                nc.vector.tensor_scalar(
                    out=po[0:rows, c0:c0 + cw], in0=exs[ci][0:rows, :],
                    scalar1=rc[0:rows, :], scalar2=None,
                    op0=mybir.AluOpType.mult,
                )
            if rows <= 64:
                nc.sync.dma_start(out=out[b, ib * P:ib * P + rows, :],
                                  in_=po[0:rows, :])
            else:
                h = NKP // 2
                nc.sync.dma_start(out=out[b, ib * P:ib * P + rows, 0:h],
                                  in_=po[0:rows, 0:h])
                nc.scalar.dma_start(out=out[b, ib * P:ib * P + rows, h:NKP],
                                    in_=po[0:rows, h:NKP])

        def s_phase(b, qTt, kTm, pad_sb):
            for ib in range(NQB):
                if b == BL - 1 and ib == NQB - 1:
                    # very last block: per-chunk psum + split exp shortens
                    # the serial tail after the final matmul
                    s_block_final(b, ib, qTt, kTm, pad_sb)
                else:
                    s_block(b, ib, qTt, kTm, pad_sb)

        # Software-pipelined one batch deep: proj(b) is EMITTED before
        # s_phase(b-1), so proj(b)'s psum-draining epilogues sit in the
        # DVE/ACT queues AHEAD of S(b-1)'s softmax tail.  With the old
        # order, proj(b)'s 5th+ chains stalled ~1-2us at every batch
        # boundary waiting for an epilogue queued behind the S stats.
        cur = load_inputs(0)
        prev = None
        for b in range(BL):
            xk, xq, pad_sb, mask_sb = cur
            ktag = "kraw" if use_mask else "kTm"
            kraw = [actpool.tile([P, NKP], BF16, tag=f"{ktag}{et}",
                                 name=f"{ktag}{et}_{b}")
                    for et in range(NET)]
            proj(xk, NKP, wk_sb, bk_sb, kraw, kchunks)
            if b == 0:
                # keep-warm fillers: xq/Wq land 0-2us after the k-proj's
                # matmuls run out (input-bandwidth bound); an idle PE here
                # risks a HAM re-throttle (observed in ~half the runs,
                # costing ~2-3us of half-clock matmuls).  Six dummy MMs
                # bridge ~1.3us of that window.
                wp2 = ppsum.tile([P, 512], F32, tag="proj", name="warm2")
                for _ in range(6):
                    nc.tensor.matmul(
                        wp2[:], lhsT=warm_in[:, 0:P], rhs=warm_in[:],
                        start=True, stop=True,
                    )
            kTm = mask_add(kraw, mask_sb, b) if use_mask else kraw
            qTt = [actpool.tile([P, NQP], BF16, tag=f"qT{et}",
                                name=f"qT{et}_{b}")
                   for et in range(NET)]
            proj(xq, NQP, wq_sb, bq_sb, qTt, qchunks)
            if b + 1 < BL:
                cur = load_inputs(b + 1)
            if prev is not None:
                s_phase(b - 1, *prev)
            prev = (qTt, kTm, pad_sb)
        s_phase(BL - 1, *prev)


def _build(NQP, NKP, use_mask):
    nc = bacc.Bacc(
        "TRN2",
        target_bir_lowering=False,
        debug=False,
        enable_asserts=False,
        num_devices=NCORES,
    )
    qT = nc.dram_tensor("qT", [BL, P, NDT * NQP], BF16, kind="ExternalInput").ap()
    kT = nc.dram_tensor("kT", [BL, P, NDT * NKP], BF16, kind="ExternalInput").ap()
    Wq = nc.dram_tensor("Wq", [P, NDT * D], BF16, kind="ExternalInput").ap()
    Wk = nc.dram_tensor("Wk", [P, NDT * D], BF16, kind="ExternalInput").ap()
    bq = nc.dram_tensor("bq", [P, NET], F32, kind="ExternalInput").ap()
    bk = nc.dram_tensor("bk", [P, NET], F32, kind="ExternalInput").ap()
    padc = nc.dram_tensor("padc", [BL, P, 1], F32, kind="ExternalInput").ap()
    maskc = None
    if use_mask:
        maskc = nc.dram_tensor(
            "maskc", [BL, P, NKP], BF16, kind="ExternalInput").ap()
    out = nc.dram_tensor("out", [BL, NQP, NKP], BF16, kind="ExternalOutput").ap()

    with tile.TileContext(nc) as tc:
        _body(tc, qT, kT, Wq, Wk, bq, bk, padc, maskc, out, NQP, NKP)
    nc.compile()
    return nc


def _get_nc(NQP, NKP, use_mask):
    key = (NQP, NKP, use_mask)
    if key not in _CACHE:
        _CACHE[key] = _build(*key)
    return _CACHE[key]


def _pad16(n):
    # 16-col granularity: tail matmuls are free-dim-priced (no LDW floor),
    # so finer padding directly cuts PE cycles (576 -> 560 for this data,
    # ~3% of the matmul work).  S-blocks still span 128 rows; a short
    # trailing row-block costs the same per column.
    return max(64, ((n + 15) // 16) * 16)


def _prep(query, key, query_mask, key_mask, Wq, bq, Wk, bk):
    bf = ml_dtypes.bfloat16
    query = np.asarray(query, dtype=np.float32)
    key = np.asarray(key, dtype=np.float32)
    qmask = np.asarray(query_mask) != 0
    kmask = np.asarray(key_mask) != 0
    qidx = [np.nonzero(qmask[g])[0] for g in range(B)]
    kidx = [np.nonzero(kmask[g])[0] for g in range(B)]
    NQP = _pad16(max(len(i) for i in qidx))
    NKP = _pad16(max(len(i) for i in kidx))
    use_mask = bool(np.any(np.asarray(bk, dtype=np.float32) != 0.0))

    # device layout [P, dt*W + col]: row p of dt-block dt holds source row
    # dt*128+p -- lets the whole tensor ship as ONE contiguous DMA
    def pack(m):  # [D, W] -> [P, NDT*W]
        W = m.shape[1]
        return m.reshape(NDT, P, W).transpose(1, 0, 2).reshape(P, NDT * W)

    Wq_b = pack(np.asarray(Wq, dtype=np.float32).astype(bf))
    Wk_b = pack(np.asarray(Wk, dtype=np.float32).astype(bf))
    # bias for feature e lives at partition e%128, column e//128
    bq_t = np.asarray(bq, dtype=np.float32).reshape(NET, P).T.copy()
    bk_t = np.asarray(bk, dtype=np.float32).reshape(NET, P).T.copy()

    in_maps = []
    for c in range(NCORES):
        qTc = np.zeros((BL, P, NDT * NQP), dtype=bf)
        kTc = np.zeros((BL, P, NDT * NKP), dtype=bf)
        padc = np.zeros((BL, P, 1), dtype=np.float32)
        imap = {"qT": qTc, "kT": kTc, "Wq": Wq_b, "Wk": Wk_b,
                "bq": bq_t, "bk": bk_t, "padc": padc}
        if use_mask:
            mk = np.zeros((BL, P, NKP), dtype=bf)
            imap["maskc"] = mk
        for b in range(BL):
            g = c * BL + b
            qi, ki = qidx[g], kidx[g]
            qt = np.zeros((D, NQP), dtype=bf)
            kt = np.zeros((D, NKP), dtype=bf)
            qt[:, :len(qi)] = query[g][qi].T.astype(bf)
            kt[:, :len(ki)] = key[g][ki].T.astype(bf)
            qTc[b] = pack(qt)
            kTc[b] = pack(kt)
            if use_mask:
                imap["maskc"][b, :, len(ki):] = bf(MASKC)
            else:
                padc[b, :, 0] = float(NKP - len(ki))
        in_maps.append(imap)
    return in_maps, qidx, kidx, NQP, NKP, use_mask


def run(query, key, query_mask, key_mask, Wq, bq, Wk, bk, **kwargs):
    """Run on hardware; returns (output, BassKernelResults)."""
    in_maps, qidx, kidx, NQP, NKP, use_mask = _prep(
        query, key, query_mask, key_mask, Wq, bq, Wk, bk)
    nc = _get_nc(NQP, NKP, use_mask)
    res = run_bass_kernel_spmd(nc, in_maps, core_ids=list(range(NCORES)),
                               **kwargs)
    full = np.zeros((B, LQ, LK), dtype=np.float32)
    for c in range(NCORES):
        oc = res.results[c]["out"]
        for b in range(BL):
            g = c * BL + b
            qi, ki = qidx[g], kidx[g]
            full[g][np.ix_(qi, ki)] = oc[b][:len(qi), :len(ki)].astype(np.float32)
    return full, res


def kernel(query, key, query_mask, key_mask, Wq, bq, Wk, bk):
    full, _ = run(query, key, query_mask, key_mask, Wq, bq, Wk, bk)
    return full

